# revision 28
# baseline (speedup 1.0000x reference)
"""GCN block (3 layers) on 8 trn2 NeuronCores, data-parallel over batch.

Math: each layer is X' = (adj + I) @ leaky_relu(X @ W).
Using ((adj+I) @ H) @ W == (adj+I) @ (H @ W), fold each layer's weight into
the previous layer's output so every layer is one big matmul against adj:

    H0 = lrelu(X0 W0)                 (tiny, on-chip)
    G0 = H0 W1 ; Z1 = adj G0 + G0 ; H1 = lrelu(Z1)
    G1 = H1 W2 ; Z2 = adj G1 + G1 ; H2 = lrelu(Z2)
    G2 = H2     ; X3 = adj G2 + G2   (final output)

fp8: adj entries are uniform in [0, 2/N], so adj^T * 2^12 fits e4m3 with
~3% per-entry rounding error that washes out to ~1e-3 output error (the
identity path, which dominates each layer's output, stays exact in
bf16/f32; bf16 is TRN2's full-rate PE path — fp16 runs at half rate). adj^T in fp8 is 16 MB -> fully resident in SBUF, read from
HBM exactly once; big matmuls run in DoubleRow fp8 (157 TF/s, 215ns per
512-column matmul with the weight load hidden).

Streaming overlap: adj^T streams in COLUMN HALVES (all 16 m-pairs' cols
[0,2048) first, then [2048,4096)), each half its own SBUF tile. Layers
0-2 run as pair-major 4-chunk half-passes. While the second half of adj
is in flight, the PE retires layer-0 first-half output, layer-1 tiny
matmuls for the first-half m-tiles, and layer-1 first-half pairs 0-7 --
hiding much of the DMA window behind compute. PSUM is two 4-slot pools
whose ring rotation matches the even/odd pass structure (a single ring
would deadlock: layer-1 second-half tiny PSUM would wait on the live
layer-1 first-half accumulators that need its output). All DMA uses the
Sync queue (hardware DGE; GpSimd falls back to slow software DGE).

Per core: 8 samples x 16 features = 128 = partition width. Layouts:
    T-layout  [c=(b,d), m]   (128 partitions, N free)
    N-layout  [m, c]         (m partitions, 128 free)
Big matmul per 512-col chunk: psum[c, chunk] accumulates
    sum_j G8[pair j].T @ adj8^T[pair j, chunk]       (DoubleRow fp8)
  + (W_next * 2^12).T @ H^T[:, chunk]                (exact identity fold)
then eviction applies lrelu with the 2^-12 unscale folded into its
constants; the final layer fuses the identity add into its f32 output
eviction instead and DMAs each chunk as it completes. The 16x16 weights
are expanded to 128x128 block-diagonal so the tiny matmuls run all 8
samples at once, 4 m-tiles per PSUM bank with one 512-wide fp8
eviction each.
"""

import numpy as np

N_FULL = 4096
D = 16
B_FULL = 64
NCORES = 8
B_CORE = B_FULL // NCORES  # 8
C = B_CORE * D  # 128 partitions
P = 128
FREE = 512
NEG_SLOPE = 0.2
ASCALE = 2.0**12

_CACHE = {}


def _leaky(nc, dest, ps, pool, width, unscale):
    """dest = leaky_relu(ps * unscale), PSUM -> SBUF.

    lrelu(z) = 0.2 z + 0.8 relu(z) with z = ps * unscale. Split across
    engines: ACT computes t = relu(0.8 * unscale * ps) (positive scale
    commutes with relu), DVE computes dest = ps * (0.2 * unscale) + t.
    Each instruction reads PSUM at most once (HW constraint).
    """
    import concourse.mybir as mybir

    t = pool.tile([P, width], mybir.dt.float32, tag="lk", name="lkt")
    nc.scalar.activation(
        t[:], ps[:], mybir.ActivationFunctionType.Relu,
        scale=(1.0 - NEG_SLOPE) * unscale,
    )
    nc.vector.scalar_tensor_tensor(
        dest, ps[:], NEG_SLOPE * unscale, t[:], mybir.AluOpType.mult,
        mybir.AluOpType.add,
    )


def _build_nc(n, free):
    """Build the Bass module (per-core program). Cached per config."""
    import concourse.bass as bass
    import concourse.mybir as mybir
    import concourse.tile as tile
    from concourse import bacc

    f32 = mybir.dt.float32
    f16 = mybir.dt.bfloat16
    f8 = mybir.dt.float8e4

    nt = n // P           # 128-row m-tiles (32)
    nch = n // free       # output column chunks (8)
    npair = n // (2 * P)  # DoubleRow m-pairs (16)
    ngrp = nt // 4        # tiny-matmul groups (8)
    nh = n // 2           # columns per half (2048)
    chh = nch // 2        # chunks per half-pass (4)

    nc = bacc.Bacc(
        "TRN2", target_bir_lowering=False, debug=False, num_devices=NCORES
    )
    xt_h = nc.dram_tensor("xt", [C, n], f16, kind="ExternalInput")
    # adj^T pre-arranged on the host as [pair, half, p, s, nh] so each
    # (pair, half) block is one contiguous [128, 2*nh] image with 4KB
    # partition rows (full-size DMA descriptors)
    at_h = nc.dram_tensor("at", [npair, 2, P, 2, nh], f8, kind="ExternalInput")
    w_h = nc.dram_tensor("wt", [7, P, P], f16, kind="ExternalInput")
    out_h = nc.dram_tensor("out", [C, n], f16, kind="ExternalOutput")

    def panel_src(j, h):
        # m-pair j, column half h: [p, s, nh] with m = j*256 + s*128 + p
        return at_h[j, h]

    def cs(ncx):
        return slice(ncx * free, (ncx + 1) * free)

    with tile.TileContext(nc) as tc:
        with (
            tc.tile_pool(name="const", bufs=1) as constp,
            tc.tile_pool(name="ht", bufs=1) as htp,
            tc.tile_pool(name="g8", bufs=1) as g8p,
            tc.tile_pool(name="outp", bufs=4) as outp,
            tc.tile_pool(name="lk", bufs=2) as lkp,
            tc.tile_pool(name="psA", bufs=4, space="PSUM") as psA,
            tc.tile_pool(name="psB", bufs=4, space="PSUM") as psB,
        ):
            pools = [psA, psB]
            w_sb = constp.tile([P, 7, P], f16)
            nc.sync.dma_start(w_sb[:], w_h[:].rearrange("w p q -> p w q"))
            xt_sb = constp.tile([C, n], f16)
            nc.sync.dma_start(xt_sb[:], xt_h[:])

            # adj^T resident in SBUF, one tile per (pair, column half);
            # all first-half panels stream before any second-half panel
            at_c = [
                [constp.tile([P, 2, nh], f8, name=f"atc{j}_{h}")
                 for h in range(2)]
                for j in range(npair)
            ]
            # split panels across the two hardware DGE queues (sync + ACT);
            # first-half panels dispatch now, second-half panels are
            # emitted after the layer-0 first-half evictions so their
            # semaphore-recycle waits never block ACT eviction work
            def dispatch_panels(h):
                for j in range(npair):
                    eng = nc.sync if j % 2 == 0 else nc.scalar
                    eng.dma_start(at_c[j][h][:], panel_src(j, h))

            dispatch_panels(0)

            # H0^T = lrelu(W0_blk.T @ X0^T)  (T-layout)
            ht0 = htp.tile([C, n], f16, name="ht0")
            for ch in range(nch):
                ps = pools[ch % 2].tile([P, free], f32, tag="ps", name="psh0")
                nc.tensor.matmul(
                    ps[:], w_sb[:, 0, :], xt_sb[:, cs(ch)],
                    start=True, stop=True,
                )
                _leaky(nc, ht0[:, cs(ch)], ps, lkp, free, 1.0)

            def tiny(g8t, ht_src, w_idx, grps, pool):
                # G8[m, c] = fp8(H^T[:, m-tile].T @ W_blk), 4 m-tiles per
                # PSUM bank, one 512-wide fp8 eviction per group
                for grp in grps:
                    psg = pool.tile([P, free], f32, tag="ps", name="psg")
                    for k in range(4):
                        mt = grp * 4 + k
                        nc.tensor.matmul(
                            psg[:, k * P:(k + 1) * P],
                            ht_src[:, mt * P:(mt + 1) * P],
                            w_sb[:, w_idx, :],
                            start=True,
                            stop=True,
                        )
                    nc.vector.tensor_copy(
                        g8t[:, grp * 2:grp * 2 + 2, :, :], psg[:]
                    )

            def half_pass(ps_l, g8t, h, pairs, fold_src, w_id, stop,
                          fresh=True):
                # accumulate the 4 chunks of column-half h for the given
                # pairs; fresh=True opens the accumulation group (via the
                # identity fold when fold_src is given, else on the first
                # pair); fresh=False continues a previous half_pass
                if fold_src is not None:
                    assert fresh
                    for k in range(chh):
                        nc.tensor.matmul(
                            ps_l[k][:],
                            w_sb[:, w_id, :],
                            fold_src[:, cs(h * chh + k)],
                            start=True,
                            stop=False,
                        )
                for j in pairs:
                    att = at_c[j][h]
                    for k in range(chh):
                        nc.tensor.matmul(
                            ps_l[k][:],
                            g8t[:, j, :, :],
                            att[:, :, k * free:(k + 1) * free],
                            perf_mode=mybir.MatmulPerfMode.DoubleRow,
                            start=(fresh and fold_src is None
                                   and j == pairs[0]),
                            stop=(stop and j == pairs[-1]),
                        )

            def accs(pool, nm):
                return [pool.tile([P, free], f32, tag="ps", name=f"{nm}{k}")
                        for k in range(chh)]

            g80 = g8p.tile([P, npair, 2, P], f8, name="g80")
            g81 = g8p.tile([P, npair, 2, P], f8, name="g81")
            g82 = g8p.tile([P, npair, 2, P], f8, name="g82")
            ht1 = htp.tile([C, n], f16, name="ht1")
            ht2 = htp.tile([C, n], f16, name="ht2")
            allp = list(range(npair))

            # ---- layer 0 first half (paced by first-half panels) ----
            tiny(g80, ht0, 1, range(ngrp), psA)
            ps0A = accs(psA, "p0a")
            half_pass(ps0A, g80, 0, allp, ht0, 4, True)
            for k in range(chh):
                _leaky(nc, ht1[:, cs(k)], ps0A[k], lkp, free, 1.0 / ASCALE)
            dispatch_panels(1)

            # ---- layer-1 work that fits inside the DMA window ----
            tiny(g81, ht1, 2, range(ngrp // 2), psA)
            ps1A = accs(psA, "p1a")
            half_pass(ps1A, g81, 0, allp[:npair // 2], ht1, 5, False)

            # ---- layer 0 second half (paced by second-half panels) ----
            ps0B = accs(psB, "p0b")
            half_pass(ps0B, g80, 1, allp, ht0, 4, True)
            for k in range(chh):
                _leaky(nc, ht1[:, cs(chh + k)], ps0B[k], lkp, free,
                       1.0 / ASCALE)

            # ---- layer 1 remainder ----
            tiny(g81, ht1, 2, range(ngrp // 2, ngrp), psB)
            half_pass(ps1A, g81, 0, allp[npair // 2:], None, 5, True,
                      fresh=False)
            for k in range(chh):
                _leaky(nc, ht2[:, cs(k)], ps1A[k], lkp, free, 1.0 / ASCALE)

            tiny(g82, ht2, 3, range(ngrp // 2), psA)

            ps1B = accs(psB, "p1b")
            half_pass(ps1B, g81, 1, allp, ht1, 5, True)
            for k in range(chh):
                _leaky(nc, ht2[:, cs(chh + k)], ps1B[k], lkp, free,
                       1.0 / ASCALE)

            tiny(g82, ht2, 3, range(ngrp // 2, ngrp), psB)

            # ---- layer 2 (two half-passes, output drains per chunk) ----
            for half in range(2):
                ps_l = accs(pools[half], "p2")
                half_pass(ps_l, g82, half, allp, None, 0, True)
                for k in range(chh):
                    ncx = half * chh + k
                    oc = outp.tile([C, free], f16, tag="oc")
                    nc.vector.scalar_tensor_tensor(
                        oc[:], ps_l[k][:], 1.0 / ASCALE, ht2[:, cs(ncx)],
                        mybir.AluOpType.mult, mybir.AluOpType.add,
                    )
                    nc.sync.dma_start(out_h[:, cs(ncx)], oc[:])

    nc.compile()
    return nc


def _get_nc(n=N_FULL, free=FREE):
    key = (n, free)
    if key not in _CACHE:
        _CACHE[key] = _build_nc(n, free)
    return _CACHE[key]


def _block_diag(w, reps):
    """(D,D) -> (reps*D, reps*D) block diagonal, f32."""
    d = w.shape[0]
    out = np.zeros((reps * d, reps * d), dtype=np.float32)
    for b in range(reps):
        out[b * d:(b + 1) * d, b * d:(b + 1) * d] = w
    return out


def prepare_inputs(x, adj, W0, W1, W2, n=N_FULL):
    """Host-side layout prep. Returns per-core input maps."""
    import ml_dtypes

    b_full = x.shape[0]
    b_core = b_full // NCORES
    c = b_core * D

    at8 = np.ascontiguousarray(
        np.asarray(adj, np.float32).T * ASCALE
    ).astype(ml_dtypes.float8_e4m3)
    # [pair, half, p, s, nh]: row m = j*256 + s*128 + p, cols split in
    # halves; each (pair, half) block is contiguous with 4KB rows
    npair = n // 256
    nh = n // 2
    at8 = np.ascontiguousarray(
        at8.reshape(npair, 2, 128, 2, nh).transpose(0, 3, 2, 1, 4)
    )

    reps = c // D
    w0 = _block_diag(np.asarray(W0, np.float32), reps)
    w1 = _block_diag(np.asarray(W1, np.float32), reps)
    w2 = _block_diag(np.asarray(W2, np.float32), reps)
    eye = np.eye(c, dtype=np.float32)
    w_all = np.stack(
        [w0, w1, w2, eye, w1 * ASCALE, w2 * ASCALE, eye * ASCALE]
    ).astype(ml_dtypes.bfloat16)

    # xt[core][b*D+d, m] = x[core*b_core + b, m, d]
    xf = np.asarray(x, np.float32)
    in_maps = []
    for core in range(NCORES):
        xs = xf[core * b_core:(core + 1) * b_core]      # (b_core, n, D)
        xt = np.ascontiguousarray(
            xs.transpose(0, 2, 1).reshape(c, n)
        ).astype(ml_dtypes.bfloat16)
        in_maps.append({"xt": xt, "at": at8, "wt": w_all})
    return in_maps


def gather_output(results, n=N_FULL, b_full=B_FULL):
    b_core = b_full // NCORES
    c = b_core * D
    out = np.empty((b_full, n, D), dtype=np.float32)
    for core in range(NCORES):
        oc = np.asarray(results[core]["out"], np.float32).reshape(b_core, D, n)
        out[core * b_core:(core + 1) * b_core] = oc.transpose(0, 2, 1)
    return out


def run(x, adj, Identity, W0, W1, W2, n=N_FULL, free=FREE, trace=False):
    from concourse.bass_utils import run_bass_kernel_spmd

    nc = _get_nc(n, free)
    in_maps = prepare_inputs(x, adj, W0, W1, W2, n)
    core_ids = list(range(NCORES))
    res = run_bass_kernel_spmd(nc, in_maps, core_ids, trace=trace)
    out = gather_output(res.results, n, x.shape[0])
    return out, res


def kernel(x, adj, Identity, W0, W1, W2):
    out, _ = run(x, adj, Identity, W0, W1, W2)
    return out


# revision 29
# speedup vs baseline: 1.0597x; 1.0597x over previous
"""GCN block (3 layers) on 8 trn2 NeuronCores, data-parallel over batch.

Math: each layer is X' = (adj + I) @ leaky_relu(X @ W).
Using ((adj+I) @ H) @ W == (adj+I) @ (H @ W), fold each layer's weight into
the previous layer's output so every layer is one big matmul against adj:

    H0 = lrelu(X0 W0)                 (tiny, on-chip)
    G0 = H0 W1 ; Z1 = adj G0 + G0 ; H1 = lrelu(Z1)
    G1 = H1 W2 ; Z2 = adj G1 + G1 ; H2 = lrelu(Z2)
    G2 = H2     ; X3 = adj G2 + G2   (final output)

fp8: adj entries are uniform in [0, 2/N], so adj^T * 2^12 fits e4m3 with
~3% per-entry rounding error that washes out to ~1e-3 output error (the
identity path, which dominates each layer's output, stays exact in
bf16/f32; bf16 is TRN2's full-rate PE path — fp16 runs at half rate). adj^T in fp8 is 16 MB -> fully resident in SBUF, read from
HBM exactly once; big matmuls run in DoubleRow fp8 (157 TF/s, 215ns per
512-column matmul with the weight load hidden).

Streaming overlap: adj^T streams in COLUMN HALVES (all 16 m-pairs' cols
[0,2048) first, then [2048,4096)), each half its own SBUF tile. Layers
0-2 run as pair-major 4-chunk half-passes. While the second half of adj
is in flight, the PE retires layer-0 first-half output, layer-1 tiny
matmuls for the first-half m-tiles, and layer-1 first-half pairs 0-7 --
hiding much of the DMA window behind compute. PSUM is two 4-slot pools
whose ring rotation matches the even/odd pass structure (a single ring
would deadlock: layer-1 second-half tiny PSUM would wait on the live
layer-1 first-half accumulators that need its output). All DMA uses the
Sync queue (hardware DGE; GpSimd falls back to slow software DGE).

Per core: 8 samples x 16 features = 128 = partition width. Layouts:
    T-layout  [c=(b,d), m]   (128 partitions, N free)
    N-layout  [m, c]         (m partitions, 128 free)
Big matmul per 512-col chunk: psum[c, chunk] accumulates
    sum_j G8[pair j].T @ adj8^T[pair j, chunk]       (DoubleRow fp8)
  + (W_next * 2^12).T @ H^T[:, chunk]                (exact identity fold)
then eviction applies lrelu with the 2^-12 unscale folded into its
constants; the final layer fuses the identity add into its f32 output
eviction instead and DMAs each chunk as it completes. The 16x16 weights
are expanded to 128x128 block-diagonal so the tiny matmuls run all 8
samples at once, 4 m-tiles per PSUM bank with one 512-wide fp8
eviction each.
"""

import numpy as np

N_FULL = 4096
D = 16
B_FULL = 64
NCORES = 8
B_CORE = B_FULL // NCORES  # 8
C = B_CORE * D  # 128 partitions
P = 128
FREE = 512
NEG_SLOPE = 0.2
ASCALE = 2.0**12

_CACHE = {}


def _leaky(nc, dest, ps, pool, width, unscale):
    """dest = leaky_relu(ps * unscale), PSUM -> SBUF.

    lrelu(z) = 0.2 z + 0.8 relu(z) with z = ps * unscale. Split across
    engines: ACT computes t = relu(0.8 * unscale * ps) (positive scale
    commutes with relu), DVE computes dest = ps * (0.2 * unscale) + t.
    Each instruction reads PSUM at most once (HW constraint).
    """
    import concourse.mybir as mybir

    t = pool.tile([P, width], mybir.dt.float32, tag="lk", name="lkt")
    nc.scalar.activation(
        t[:], ps[:], mybir.ActivationFunctionType.Relu,
        scale=(1.0 - NEG_SLOPE) * unscale,
    )
    nc.vector.scalar_tensor_tensor(
        dest, ps[:], NEG_SLOPE * unscale, t[:], mybir.AluOpType.mult,
        mybir.AluOpType.add,
    )


def _build_nc(n, free):
    """Build the Bass module (per-core program). Cached per config."""
    import concourse.bass as bass
    import concourse.mybir as mybir
    import concourse.tile as tile
    from concourse import bacc

    f32 = mybir.dt.float32
    f16 = mybir.dt.bfloat16
    f8 = mybir.dt.float8e4

    nt = n // P           # 128-row m-tiles (32)
    nch = n // free       # output column chunks (8)
    npair = n // (2 * P)  # DoubleRow m-pairs (16)
    ngrp = nt // 4        # tiny-matmul groups (8)
    nh = n // 2           # columns per half (2048)
    chh = nch // 2        # chunks per half-pass (4)

    nc = bacc.Bacc(
        "TRN2", target_bir_lowering=False, debug=False, num_devices=NCORES
    )
    xt_h = nc.dram_tensor("xt", [C, n], f16, kind="ExternalInput")
    # adj^T pre-arranged on the host as [pair, half, p, s, nh] so each
    # (pair, half) block is one contiguous [128, 2*nh] image with 4KB
    # partition rows (full-size DMA descriptors)
    at_h = nc.dram_tensor("at", [npair, 2, P, 2, nh], f8, kind="ExternalInput")
    w_h = nc.dram_tensor("wt", [7, P, P], f16, kind="ExternalInput")
    out_h = nc.dram_tensor("out", [C, n], f16, kind="ExternalOutput")

    def panel_src(j, h):
        # m-pair j, column half h: [p, s, nh] with m = j*256 + s*128 + p
        return at_h[j, h]

    def cs(ncx):
        return slice(ncx * free, (ncx + 1) * free)

    with tile.TileContext(nc) as tc:
        with (
            tc.tile_pool(name="const", bufs=1) as constp,
            tc.tile_pool(name="ht", bufs=1) as htp,
            tc.tile_pool(name="g8", bufs=1) as g8p,
            tc.tile_pool(name="outp", bufs=4) as outp,
            tc.tile_pool(name="lk", bufs=2) as lkp,
            tc.tile_pool(name="psA", bufs=4, space="PSUM") as psA,
            tc.tile_pool(name="psB", bufs=4, space="PSUM") as psB,
        ):
            pools = [psA, psB]
            w_sb = constp.tile([P, 7, P], f16)
            nc.sync.dma_start(w_sb[:], w_h[:].rearrange("w p q -> p w q"))
            # x^T split across both hardware DGE queues so neither
            # stream straggles and H0's input lands early
            xt_sb = constp.tile([C, n], f16)
            nc.sync.dma_start(xt_sb[:, 0:nh], xt_h[:, 0:nh])
            nc.scalar.dma_start(xt_sb[:, nh:], xt_h[:, nh:])

            # adj^T resident in SBUF, one tile per (pair, column half);
            # all first-half panels stream before any second-half panel
            at_c = [
                [constp.tile([P, 2, nh], f8, name=f"atc{j}_{h}")
                 for h in range(2)]
                for j in range(npair)
            ]
            # split panels across the two hardware DGE queues (sync + ACT);
            # first-half panels dispatch now, second-half panels are
            # emitted after the layer-0 first-half evictions so their
            # semaphore-recycle waits never block ACT eviction work
            def dispatch_panels(h):
                for j in range(npair):
                    eng = nc.sync if j % 2 == 0 else nc.scalar
                    eng.dma_start(at_c[j][h][:], panel_src(j, h))

            dispatch_panels(0)

            # H0^T = lrelu(W0_blk.T @ X0^T)  (T-layout)
            ht0 = htp.tile([C, n], f16, name="ht0")
            for ch in range(nch):
                ps = pools[ch % 2].tile([P, free], f32, tag="ps", name="psh0")
                nc.tensor.matmul(
                    ps[:], w_sb[:, 0, :], xt_sb[:, cs(ch)],
                    start=True, stop=True,
                )
                _leaky(nc, ht0[:, cs(ch)], ps, lkp, free, 1.0)

            def tiny(g8t, ht_src, w_idx, grps, pool):
                # G8[m, c] = fp8(H^T[:, m-tile].T @ W_blk), 4 m-tiles per
                # PSUM bank, one 512-wide fp8 eviction per group
                for grp in grps:
                    psg = pool.tile([P, free], f32, tag="ps", name="psg")
                    for k in range(4):
                        mt = grp * 4 + k
                        nc.tensor.matmul(
                            psg[:, k * P:(k + 1) * P],
                            ht_src[:, mt * P:(mt + 1) * P],
                            w_sb[:, w_idx, :],
                            start=True,
                            stop=True,
                        )
                    nc.vector.tensor_copy(
                        g8t[:, grp * 2:grp * 2 + 2, :, :], psg[:]
                    )

            def half_pass(ps_l, g8t, h, pairs, fold_src, w_id, stop,
                          fresh=True):
                # accumulate the 4 chunks of column-half h for the given
                # pairs; fresh=True opens the accumulation group (via the
                # identity fold when fold_src is given, else on the first
                # pair); fresh=False continues a previous half_pass
                if fold_src is not None:
                    assert fresh
                    for k in range(chh):
                        nc.tensor.matmul(
                            ps_l[k][:],
                            w_sb[:, w_id, :],
                            fold_src[:, cs(h * chh + k)],
                            start=True,
                            stop=False,
                        )
                for j in pairs:
                    att = at_c[j][h]
                    for k in range(chh):
                        nc.tensor.matmul(
                            ps_l[k][:],
                            g8t[:, j, :, :],
                            att[:, :, k * free:(k + 1) * free],
                            perf_mode=mybir.MatmulPerfMode.DoubleRow,
                            start=(fresh and fold_src is None
                                   and j == pairs[0]),
                            stop=(stop and j == pairs[-1]),
                        )

            def accs(pool, nm):
                return [pool.tile([P, free], f32, tag="ps", name=f"{nm}{k}")
                        for k in range(chh)]

            g80 = g8p.tile([P, npair, 2, P], f8, name="g80")
            g81 = g8p.tile([P, npair, 2, P], f8, name="g81")
            g82 = g8p.tile([P, npair, 2, P], f8, name="g82")
            ht1 = htp.tile([C, n], f16, name="ht1")
            ht2 = htp.tile([C, n], f16, name="ht2")
            allp = list(range(npair))

            # ---- layer 0 first half (paced by first-half panels) ----
            tiny(g80, ht0, 1, range(ngrp), psA)
            ps0A = accs(psA, "p0a")
            half_pass(ps0A, g80, 0, allp, ht0, 4, True)
            for k in range(chh):
                _leaky(nc, ht1[:, cs(k)], ps0A[k], lkp, free, 1.0 / ASCALE)
            dispatch_panels(1)

            # ---- layer-1 work that fits inside the DMA window ----
            tiny(g81, ht1, 2, range(ngrp // 2), psA)
            ps1A = accs(psA, "p1a")
            half_pass(ps1A, g81, 0, allp[:npair // 2], ht1, 5, False)

            # ---- layer 0 second half (paced by second-half panels) ----
            ps0B = accs(psB, "p0b")
            half_pass(ps0B, g80, 1, allp, ht0, 4, True)
            for k in range(chh):
                _leaky(nc, ht1[:, cs(chh + k)], ps0B[k], lkp, free,
                       1.0 / ASCALE)

            # ---- layer 1 remainder ----
            tiny(g81, ht1, 2, range(ngrp // 2, ngrp), psB)
            half_pass(ps1A, g81, 0, allp[npair // 2:], None, 5, True,
                      fresh=False)
            for k in range(chh):
                _leaky(nc, ht2[:, cs(k)], ps1A[k], lkp, free, 1.0 / ASCALE)

            tiny(g82, ht2, 3, range(ngrp // 2), psA)

            ps1B = accs(psB, "p1b")
            half_pass(ps1B, g81, 1, allp, ht1, 5, True)
            for k in range(chh):
                _leaky(nc, ht2[:, cs(chh + k)], ps1B[k], lkp, free,
                       1.0 / ASCALE)

            tiny(g82, ht2, 3, range(ngrp // 2, ngrp), psB)

            # ---- layer 2 (two half-passes, output drains per chunk) ----
            for half in range(2):
                ps_l = accs(pools[half], "p2")
                half_pass(ps_l, g82, half, allp, None, 0, True)
                for k in range(chh):
                    ncx = half * chh + k
                    oc = outp.tile([C, free], f16, tag="oc")
                    nc.vector.scalar_tensor_tensor(
                        oc[:], ps_l[k][:], 1.0 / ASCALE, ht2[:, cs(ncx)],
                        mybir.AluOpType.mult, mybir.AluOpType.add,
                    )
                    nc.sync.dma_start(out_h[:, cs(ncx)], oc[:])

    nc.compile()
    return nc


def _get_nc(n=N_FULL, free=FREE):
    key = (n, free)
    if key not in _CACHE:
        _CACHE[key] = _build_nc(n, free)
    return _CACHE[key]


def _block_diag(w, reps):
    """(D,D) -> (reps*D, reps*D) block diagonal, f32."""
    d = w.shape[0]
    out = np.zeros((reps * d, reps * d), dtype=np.float32)
    for b in range(reps):
        out[b * d:(b + 1) * d, b * d:(b + 1) * d] = w
    return out


def prepare_inputs(x, adj, W0, W1, W2, n=N_FULL):
    """Host-side layout prep. Returns per-core input maps."""
    import ml_dtypes

    b_full = x.shape[0]
    b_core = b_full // NCORES
    c = b_core * D

    at8 = np.ascontiguousarray(
        np.asarray(adj, np.float32).T * ASCALE
    ).astype(ml_dtypes.float8_e4m3)
    # [pair, half, p, s, nh]: row m = j*256 + s*128 + p, cols split in
    # halves; each (pair, half) block is contiguous with 4KB rows
    npair = n // 256
    nh = n // 2
    at8 = np.ascontiguousarray(
        at8.reshape(npair, 2, 128, 2, nh).transpose(0, 3, 2, 1, 4)
    )

    reps = c // D
    w0 = _block_diag(np.asarray(W0, np.float32), reps)
    w1 = _block_diag(np.asarray(W1, np.float32), reps)
    w2 = _block_diag(np.asarray(W2, np.float32), reps)
    eye = np.eye(c, dtype=np.float32)
    w_all = np.stack(
        [w0, w1, w2, eye, w1 * ASCALE, w2 * ASCALE, eye * ASCALE]
    ).astype(ml_dtypes.bfloat16)

    # xt[core][b*D+d, m] = x[core*b_core + b, m, d]
    xf = np.asarray(x, np.float32)
    in_maps = []
    for core in range(NCORES):
        xs = xf[core * b_core:(core + 1) * b_core]      # (b_core, n, D)
        xt = np.ascontiguousarray(
            xs.transpose(0, 2, 1).reshape(c, n)
        ).astype(ml_dtypes.bfloat16)
        in_maps.append({"xt": xt, "at": at8, "wt": w_all})
    return in_maps


def gather_output(results, n=N_FULL, b_full=B_FULL):
    b_core = b_full // NCORES
    c = b_core * D
    out = np.empty((b_full, n, D), dtype=np.float32)
    for core in range(NCORES):
        oc = np.asarray(results[core]["out"], np.float32).reshape(b_core, D, n)
        out[core * b_core:(core + 1) * b_core] = oc.transpose(0, 2, 1)
    return out


def run(x, adj, Identity, W0, W1, W2, n=N_FULL, free=FREE, trace=False):
    from concourse.bass_utils import run_bass_kernel_spmd

    nc = _get_nc(n, free)
    in_maps = prepare_inputs(x, adj, W0, W1, W2, n)
    core_ids = list(range(NCORES))
    res = run_bass_kernel_spmd(nc, in_maps, core_ids, trace=trace)
    out = gather_output(res.results, n, x.shape[0])
    return out, res


def kernel(x, adj, Identity, W0, W1, W2):
    out, _ = run(x, adj, Identity, W0, W1, W2)
    return out


# revision 30
# speedup vs baseline: 1.0879x; 1.0266x over previous
"""GCN block (3 layers) on 8 trn2 NeuronCores, data-parallel over batch.

Math: each layer is X' = (adj + I) @ leaky_relu(X @ W).
Using ((adj+I) @ H) @ W == (adj+I) @ (H @ W), fold each layer's weight into
the previous layer's output so every layer is one big matmul against adj:

    H0 = lrelu(X0 W0)                 (tiny, on-chip)
    G0 = H0 W1 ; Z1 = adj G0 + G0 ; H1 = lrelu(Z1)
    G1 = H1 W2 ; Z2 = adj G1 + G1 ; H2 = lrelu(Z2)
    G2 = H2     ; X3 = adj G2 + G2   (final output)

fp8: adj entries are uniform in [0, 2/N], so adj^T * 2^12 fits e4m3 with
~3% per-entry rounding error that washes out to ~1e-3 output error (the
identity path, which dominates each layer's output, stays exact in
bf16/f32; bf16 is TRN2's full-rate PE path — fp16 runs at half rate). adj^T in fp8 is 16 MB -> fully resident in SBUF, read from
HBM exactly once; big matmuls run in DoubleRow fp8 (157 TF/s, 215ns per
512-column matmul with the weight load hidden).

Streaming overlap: adj^T streams in COLUMN HALVES (all 16 m-pairs' cols
[0,2048) first, then [2048,4096)), each half its own SBUF tile. Layers
0-2 run as pair-major 4-chunk half-passes. While the second half of adj
is in flight, the PE retires layer-0 first-half output, layer-1 tiny
matmuls for the first-half m-tiles, and layer-1 first-half pairs 0-7 --
hiding much of the DMA window behind compute. PSUM is two 4-slot pools
whose ring rotation matches the even/odd pass structure (a single ring
would deadlock: layer-1 second-half tiny PSUM would wait on the live
layer-1 first-half accumulators that need its output). All DMA uses the
Sync queue (hardware DGE; GpSimd falls back to slow software DGE).

Per core: 8 samples x 16 features = 128 = partition width. Layouts:
    T-layout  [c=(b,d), m]   (128 partitions, N free)
    N-layout  [m, c]         (m partitions, 128 free)
Big matmul per 512-col chunk: psum[c, chunk] accumulates
    sum_j G8[pair j].T @ adj8^T[pair j, chunk]       (DoubleRow fp8)
  + (W_next * 2^12).T @ H^T[:, chunk]                (exact identity fold)
then eviction applies lrelu with the 2^-12 unscale folded into its
constants; the final layer fuses the identity add into its f32 output
eviction instead and DMAs each chunk as it completes. The 16x16 weights
are expanded to 128x128 block-diagonal so the tiny matmuls run all 8
samples at once, 4 m-tiles per PSUM bank with one 512-wide fp8
eviction each.
"""

import numpy as np

N_FULL = 4096
D = 16
B_FULL = 64
NCORES = 8
B_CORE = B_FULL // NCORES  # 8
C = B_CORE * D  # 128 partitions
P = 128
FREE = 512
NEG_SLOPE = 0.2
ASCALE = 2.0**12

_CACHE = {}


def _leaky(nc, dest, ps, pool, width, unscale):
    """dest = leaky_relu(ps * unscale), PSUM -> SBUF.

    lrelu(z) = 0.2 z + 0.8 relu(z) with z = ps * unscale. Split across
    engines: ACT computes t = relu(0.8 * unscale * ps) (positive scale
    commutes with relu), DVE computes dest = ps * (0.2 * unscale) + t.
    Each instruction reads PSUM at most once (HW constraint).
    """
    import concourse.mybir as mybir

    t = pool.tile([P, width], mybir.dt.float32, tag="lk", name="lkt")
    nc.scalar.activation(
        t[:], ps[:], mybir.ActivationFunctionType.Relu,
        scale=(1.0 - NEG_SLOPE) * unscale,
    )
    nc.vector.scalar_tensor_tensor(
        dest, ps[:], NEG_SLOPE * unscale, t[:], mybir.AluOpType.mult,
        mybir.AluOpType.add,
    )


def _build_nc(n, free):
    """Build the Bass module (per-core program). Cached per config."""
    import concourse.bass as bass
    import concourse.mybir as mybir
    import concourse.tile as tile
    from concourse import bacc

    f32 = mybir.dt.float32
    f16 = mybir.dt.bfloat16
    f8 = mybir.dt.float8e4

    nt = n // P           # 128-row m-tiles (32)
    nch = n // free       # output column chunks (8)
    npair = n // (2 * P)  # DoubleRow m-pairs (16)
    ngrp = nt // 4        # tiny-matmul groups (8)
    nh = n // 2           # columns per half (2048)
    chh = nch // 2        # chunks per half-pass (4)

    nc = bacc.Bacc(
        "TRN2", target_bir_lowering=False, debug=False, num_devices=NCORES
    )
    xt_h = nc.dram_tensor("xt", [C, n], f16, kind="ExternalInput")
    # adj^T pre-arranged on the host as [pair, half, p, s, nh] so each
    # (pair, half) block is one contiguous [128, 2*nh] image with 4KB
    # partition rows (full-size DMA descriptors)
    at_h = nc.dram_tensor("at", [npair, 2, P, 2, nh], f8, kind="ExternalInput")
    w_h = nc.dram_tensor("wt", [7, P, P], f16, kind="ExternalInput")
    out_h = nc.dram_tensor("out", [C, n], f16, kind="ExternalOutput")

    def panel_src(j, h):
        # m-pair j, column half h: [p, s, nh] with m = j*256 + s*128 + p
        return at_h[j, h]

    def cs(ncx):
        return slice(ncx * free, (ncx + 1) * free)

    with tile.TileContext(nc) as tc:
        with (
            tc.tile_pool(name="const", bufs=1) as constp,
            tc.tile_pool(name="ht", bufs=1) as htp,
            tc.tile_pool(name="g8", bufs=1) as g8p,
            tc.tile_pool(name="outp", bufs=4) as outp,
            tc.tile_pool(name="lk", bufs=2) as lkp,
            tc.tile_pool(name="psA", bufs=4, space="PSUM") as psA,
            tc.tile_pool(name="psB", bufs=4, space="PSUM") as psB,
        ):
            pools = [psA, psB]
            w_sb = constp.tile([P, 7, P], f16)
            nc.sync.dma_start(w_sb[:], w_h[:].rearrange("w p q -> p w q"))
            # x^T split across both hardware DGE queues so neither
            # stream straggles and H0's input lands early
            xt_sb = constp.tile([C, n], f16)
            nc.sync.dma_start(xt_sb[:, 0:nh], xt_h[:, 0:nh])
            nc.scalar.dma_start(xt_sb[:, nh:], xt_h[:, nh:])

            # adj^T resident in SBUF, one tile per (pair, column half);
            # all first-half panels stream before any second-half panel
            at_c = [
                [constp.tile([P, 2, nh], f8, name=f"atc{j}_{h}")
                 for h in range(2)]
                for j in range(npair)
            ]
            # split panels across the two hardware DGE queues (sync + ACT);
            # first-half panels dispatch now, second-half panels are
            # emitted after the layer-0 first-half evictions so their
            # semaphore-recycle waits never block ACT eviction work
            def dispatch_panels(h):
                for j in range(npair):
                    eng = nc.sync if j % 2 == 0 else nc.scalar
                    eng.dma_start(at_c[j][h][:], panel_src(j, h))

            dispatch_panels(0)

            # H0^T = lrelu(W0_blk.T @ X0^T)  (T-layout)
            ht0 = htp.tile([C, n], f16, name="ht0")
            for ch in range(nch):
                ps = pools[ch % 2].tile([P, free], f32, tag="ps", name="psh0")
                nc.tensor.matmul(
                    ps[:], w_sb[:, 0, :], xt_sb[:, cs(ch)],
                    start=True, stop=True,
                )
                _leaky(nc, ht0[:, cs(ch)], ps, lkp, free, 1.0)

            def tiny(g8t, ht_src, w_idx, grps, pool):
                # G8[m, c] = fp8(H^T[:, m-tile].T @ W_blk), 4 m-tiles per
                # PSUM bank, one 512-wide fp8 eviction per group
                for grp in grps:
                    psg = pool.tile([P, free], f32, tag="ps", name="psg")
                    for k in range(4):
                        mt = grp * 4 + k
                        nc.tensor.matmul(
                            psg[:, k * P:(k + 1) * P],
                            ht_src[:, mt * P:(mt + 1) * P],
                            w_sb[:, w_idx, :],
                            start=True,
                            stop=True,
                        )
                    nc.vector.tensor_copy(
                        g8t[:, grp * 2:grp * 2 + 2, :, :], psg[:]
                    )

            def half_pass(ps_l, g8t, h, pairs, fold_src, w_id, stop,
                          fresh=True):
                # accumulate the 4 chunks of column-half h for the given
                # pairs; fresh=True opens the accumulation group (via the
                # identity fold when fold_src is given, else on the first
                # pair); fresh=False continues a previous half_pass
                if fold_src is not None:
                    assert fresh
                    for k in range(chh):
                        nc.tensor.matmul(
                            ps_l[k][:],
                            w_sb[:, w_id, :],
                            fold_src[:, cs(h * chh + k)],
                            start=True,
                            stop=False,
                        )
                for j in pairs:
                    att = at_c[j][h]
                    for k in range(chh):
                        nc.tensor.matmul(
                            ps_l[k][:],
                            g8t[:, j, :, :],
                            att[:, :, k * free:(k + 1) * free],
                            perf_mode=mybir.MatmulPerfMode.DoubleRow,
                            start=(fresh and fold_src is None
                                   and j == pairs[0]),
                            stop=(stop and j == pairs[-1]),
                        )

            def accs(pool, nm):
                return [pool.tile([P, free], f32, tag="ps", name=f"{nm}{k}")
                        for k in range(chh)]

            g80 = g8p.tile([P, npair, 2, P], f8, name="g80")
            g81 = g8p.tile([P, npair, 2, P], f8, name="g81")
            g82 = g8p.tile([P, npair, 2, P], f8, name="g82")
            ht1 = htp.tile([C, n], f16, name="ht1")
            ht2 = htp.tile([C, n], f16, name="ht2")
            allp = list(range(npair))

            # ---- layer 0 first half (paced by first-half panels) ----
            tiny(g80, ht0, 1, range(ngrp), psA)
            ps0A = accs(psA, "p0a")
            half_pass(ps0A, g80, 0, allp, ht0, 4, True)
            for k in range(chh):
                _leaky(nc, ht1[:, cs(k)], ps0A[k], lkp, free, 1.0 / ASCALE)
            dispatch_panels(1)

            # ---- layer-1 work that fits inside the DMA window ----
            tiny(g81, ht1, 2, range(ngrp // 2), psA)
            ps1A = accs(psA, "p1a")
            half_pass(ps1A, g81, 0, allp[:npair // 2], ht1, 5, False)

            # ---- layer 0 second half (paced by second-half panels) ----
            ps0B = accs(psB, "p0b")
            half_pass(ps0B, g80, 1, allp, ht0, 4, True)
            for k in range(chh):
                _leaky(nc, ht1[:, cs(chh + k)], ps0B[k], lkp, free,
                       1.0 / ASCALE)

            # ---- layer 1 remainder ----
            tiny(g81, ht1, 2, range(ngrp // 2, ngrp), psB)
            half_pass(ps1A, g81, 0, allp[npair // 2:], None, 5, True,
                      fresh=False)
            for k in range(chh):
                _leaky(nc, ht2[:, cs(k)], ps1A[k], lkp, free, 1.0 / ASCALE)

            tiny(g82, ht2, 3, range(ngrp // 2), psA)

            ps1B = accs(psB, "p1b")
            half_pass(ps1B, g81, 1, allp, ht1, 5, True)
            for k in range(chh):
                _leaky(nc, ht2[:, cs(chh + k)], ps1B[k], lkp, free,
                       1.0 / ASCALE)

            tiny(g82, ht2, 3, range(ngrp // 2, ngrp), psB)

            # ---- layer 2: first half-pass, then the second half as two
            # quarter-passes so the final evictions and output DMA hide
            # under the last quarter's matmuls ----
            def l2_pass(pool, chunks, nm):
                ps_l = [pool.tile([P, free], f32, tag="ps", name=f"{nm}{k}")
                        for k in range(len(chunks))]
                for j in allp:
                    att_h1 = at_c[j]
                    for k, ncx in enumerate(chunks):
                        h = ncx // chh
                        kk = ncx % chh
                        nc.tensor.matmul(
                            ps_l[k][:],
                            g82[:, j, :, :],
                            at_c[j][h][:, :, kk * free:(kk + 1) * free],
                            perf_mode=mybir.MatmulPerfMode.DoubleRow,
                            start=(j == 0),
                            stop=(j == npair - 1),
                        )
                for k, ncx in enumerate(chunks):
                    oc = outp.tile([C, free], f16, tag="oc")
                    nc.vector.scalar_tensor_tensor(
                        oc[:], ps_l[k][:], 1.0 / ASCALE, ht2[:, cs(ncx)],
                        mybir.AluOpType.mult, mybir.AluOpType.add,
                    )
                    nc.sync.dma_start(out_h[:, cs(ncx)], oc[:])

            l2_pass(psA, [0, 1, 2, 3], "p2a")
            l2_pass(psB, [4, 5], "p2b")
            l2_pass(psB, [6, 7], "p2c")

    nc.compile()
    return nc


def _get_nc(n=N_FULL, free=FREE):
    key = (n, free)
    if key not in _CACHE:
        _CACHE[key] = _build_nc(n, free)
    return _CACHE[key]


def _block_diag(w, reps):
    """(D,D) -> (reps*D, reps*D) block diagonal, f32."""
    d = w.shape[0]
    out = np.zeros((reps * d, reps * d), dtype=np.float32)
    for b in range(reps):
        out[b * d:(b + 1) * d, b * d:(b + 1) * d] = w
    return out


def prepare_inputs(x, adj, W0, W1, W2, n=N_FULL):
    """Host-side layout prep. Returns per-core input maps."""
    import ml_dtypes

    b_full = x.shape[0]
    b_core = b_full // NCORES
    c = b_core * D

    at8 = np.ascontiguousarray(
        np.asarray(adj, np.float32).T * ASCALE
    ).astype(ml_dtypes.float8_e4m3)
    # [pair, half, p, s, nh]: row m = j*256 + s*128 + p, cols split in
    # halves; each (pair, half) block is contiguous with 4KB rows
    npair = n // 256
    nh = n // 2
    at8 = np.ascontiguousarray(
        at8.reshape(npair, 2, 128, 2, nh).transpose(0, 3, 2, 1, 4)
    )

    reps = c // D
    w0 = _block_diag(np.asarray(W0, np.float32), reps)
    w1 = _block_diag(np.asarray(W1, np.float32), reps)
    w2 = _block_diag(np.asarray(W2, np.float32), reps)
    eye = np.eye(c, dtype=np.float32)
    w_all = np.stack(
        [w0, w1, w2, eye, w1 * ASCALE, w2 * ASCALE, eye * ASCALE]
    ).astype(ml_dtypes.bfloat16)

    # xt[core][b*D+d, m] = x[core*b_core + b, m, d]
    xf = np.asarray(x, np.float32)
    in_maps = []
    for core in range(NCORES):
        xs = xf[core * b_core:(core + 1) * b_core]      # (b_core, n, D)
        xt = np.ascontiguousarray(
            xs.transpose(0, 2, 1).reshape(c, n)
        ).astype(ml_dtypes.bfloat16)
        in_maps.append({"xt": xt, "at": at8, "wt": w_all})
    return in_maps


def gather_output(results, n=N_FULL, b_full=B_FULL):
    b_core = b_full // NCORES
    c = b_core * D
    out = np.empty((b_full, n, D), dtype=np.float32)
    for core in range(NCORES):
        oc = np.asarray(results[core]["out"], np.float32).reshape(b_core, D, n)
        out[core * b_core:(core + 1) * b_core] = oc.transpose(0, 2, 1)
    return out


def run(x, adj, Identity, W0, W1, W2, n=N_FULL, free=FREE, trace=False):
    from concourse.bass_utils import run_bass_kernel_spmd

    nc = _get_nc(n, free)
    in_maps = prepare_inputs(x, adj, W0, W1, W2, n)
    core_ids = list(range(NCORES))
    res = run_bass_kernel_spmd(nc, in_maps, core_ids, trace=trace)
    out = gather_output(res.results, n, x.shape[0])
    return out, res


def kernel(x, adj, Identity, W0, W1, W2):
    out, _ = run(x, adj, Identity, W0, W1, W2)
    return out
